# revision 35
# baseline (speedup 1.0000x reference)
"""DeepseekV3 MLA decode attention (B=32, H=128, q_len=1, T=4096) on 8 trn2 NeuronCores.

Strategy: batch-parallel over the 8 cores (4 batches/core). Per core, the full
absorbed-MLA decode runs on device:
  - absorb:   q_lat[b,h,c] = q_nope[b,h,:] @ w_ukv[h]   (PE, quadrant-packed),
              then DRAM-roundtrip to [b,h,c] + PE transpose -> lq[b][c,h] fp8
  - scores:   s[h,t] = lq^T kT  (fp8 x fp8 on PE, fp32 PSUM)
  - softmax:  no-max-pass exp (logits are tiny: |s*scale| < ~1), fused
              denominator accumulation on ACT, reciprocal on DVE
  - value:    x[h,c] = probs @ ckv (probs transposed on PE, fp16)
  - out-proj: out[b,h,v] = x[b,h,:] @ w_v[h]^T          (PE, quadrant-packed)
The score-side operands (kT cache, w_ukv, k_pe^T) stream in fp8e4m3 - softmax's
tiny-logit regime makes score-side quantization error negligible. The value
path (ckv, probs, w_v) stays fp16. Host prep is layout-only (RoPE of the
single query token, cache transpose, weight reshapes); all matmul FLOPs are
on device.
"""

import functools

import numpy as np

B, H, NOPE, ROPE, V, LORA = 32, 128, 128, 64, 128, 512
CACHE_LEN, START_POS = 4096, 4095
T = START_POS + 1            # 4096 keys after cache append
CR = LORA + ROPE             # 576 rows of the transposed key matrix
SCALE = float((NOPE + ROPE) ** -0.5)
N_CORES = 8
BL = B // N_CORES            # 4 batches per core
NT = T // 512                # 8 score chunks of 512
NTC = T // 128               # 32 value chunks of 128
NKC = LORA // 128            # 4 latent c-chunks of 128
NJ = T // 1024               # 4 DMA tiles of 1024 keys


def _interleave_to_half(x):
    *lead, d = x.shape
    return x.reshape(*lead, d // 2, 2).swapaxes(-1, -2).reshape(*lead, d)


def _rotate_half(x):
    d = x.shape[-1]
    return np.concatenate([-x[..., d // 2:], x[..., :d // 2]], axis=-1)


def _host_rope(q_pe, k_pe, cos, sin, position_ids):
    kv_seq_len = 1
    c = cos[:kv_seq_len][position_ids][:, None]  # [B,1,1,64]
    s = sin[:kv_seq_len][position_ids][:, None]
    q = _interleave_to_half(q_pe)
    k = _interleave_to_half(k_pe)
    return q * c + _rotate_half(q) * s, k * c + _rotate_half(k) * s


@functools.lru_cache(maxsize=4)
def _build_program(reps=1, bf16=True):
    import concourse.bacc as bacc
    import concourse.mybir as mybir
    import concourse.tile as tile
    from concourse.masks import make_identity

    f32 = mybir.dt.float32
    dk = mybir.dt.float16
    f8 = mybir.dt.float8e4
    nc = bacc.Bacc("TRN2", target_bir_lowering=False, debug=False)

    # ---- DRAM I/O ----
    k8 = nc.dram_tensor("k8", [BL, CR, T], f8, kind="ExternalInput").ap()
    v_c = nc.dram_tensor("v_c", [BL, CACHE_LEN, LORA], dk, kind="ExternalInput").ap()
    newkv = nc.dram_tensor("newkv", [BL, LORA], dk, kind="ExternalInput").ap()
    qn_t = nc.dram_tensor("qn_t", [NOPE, 32, 4, 32], dk, kind="ExternalInput").ap()
    qpe_t = nc.dram_tensor("qpe_t", [BL, ROPE, H], dk, kind="ExternalInput").ap()
    wu8 = nc.dram_tensor("wu8", [8, NOPE, 4, 4, LORA], f8, kind="ExternalInput").ap()
    wv2 = nc.dram_tensor("wv2", [16, 128, 8, NKC, V], dk, kind="ExternalInput").ap()
    out_d = nc.dram_tensor("out_d", [BL, H, V], f32, kind="ExternalOutput").ap()

    from contextlib import ExitStack

    with tile.TileContext(nc) as tc, ExitStack() as st0:
        constp = st0.enter_context(tc.tile_pool(name="const", bufs=1))
        dramp = st0.enter_context(tc.tile_pool(name="dram", bufs=1, space="DRAM"))

        identity = constp.tile([128, 128], dk)
        make_identity(nc, identity)
        identity8 = constp.tile([128, 128], f8)
        make_identity(nc, identity8)

        ql_d = dramp.tile([BL, H, LORA], dk)

        for _rep in range(reps):
            _emit_body(nc, tc, mybir, f32, dk, f8, _rep,
                       identity, identity8, qn_t, k8, v_c, newkv, qpe_t,
                       wu8, wv2, ql_d, out_d)

    nc.compile()
    return nc


def _emit_body(nc, tc, mybir, f32, dk, f8, rep,
               identity, identity8, qn_sb_d, k8, v_c, newkv, qpe_t, wu8, wv2,
               ql_d, out_d):
    from contextlib import ExitStack

    def pname(s):
        return f"{s}_r{rep}"

    with ExitStack() as stC:
        constp = stC.enter_context(tc.tile_pool(name=pname("perrep"), bufs=1))

        # Phase B/D pools first: their SBUF zones must not overlap phase-A
        # pools so cache prefetch DMAs start at t=0.
        stB = ExitStack()
        ktP = stB.enter_context(tc.tile_pool(name=pname("ktP"), bufs=2))
        roP = stB.enter_context(tc.tile_pool(name=pname("roP"), bufs=2))
        vtP = stB.enter_context(tc.tile_pool(name=pname("vtP"), bufs=4))
        prP = stB.enter_context(tc.tile_pool(name=pname("prP"), bufs=3))
        stP = stB.enter_context(tc.tile_pool(name=pname("stP"), bufs=2))
        pTP = stB.enter_context(tc.tile_pool(name=pname("pTP"), bufs=2))
        xP = stB.enter_context(tc.tile_pool(name=pname("xP"), bufs=1))
        xtP = stB.enter_context(tc.tile_pool(name=pname("xtP"), bufs=1))
        outP = stB.enter_context(tc.tile_pool(name=pname("outP"), bufs=1))
        psS = stB.enter_context(tc.tile_pool(name=pname("psS"), bufs=4, space="PSUM"))
        psT = stB.enter_context(tc.tile_pool(name=pname("psT"), bufs=2, space="PSUM"))
        psX = stB.enter_context(tc.tile_pool(name=pname("psX"), bufs=2, space="PSUM"))

        # kt/ro prefetch triggers (gpsimd stream, no upstream deps)
        kts, ros = [], []
        for b in range(BL):
            kt = ktP.tile([128, NKC, T], f8)
            src_ = k8[b, 0:LORA, :].rearrange("(k p) t -> p k t", p=128)
            for k in range(NKC):
                nc.gpsimd.dma_start(out=kt[:, k, :], in_=src_[:, k, :])
            ro = roP.tile([ROPE, T], f8)
            nc.gpsimd.dma_start(out=ro, in_=k8[b, LORA:CR, :])
            kts.append(kt)
            ros.append(ro)

        # ---------- Phase A: absorb q_nope @ w_ukv -> lq[b] = [c, h] ----------
        stA = ExitStack()
        wuP = stA.enter_context(tc.tile_pool(name=pname("wuP"), bufs=4))
        qlP = stA.enter_context(tc.tile_pool(name=pname("qlP"), bufs=1))

        qn_sb = qlP.tile([NOPE, 32, 4, 32], dk)
        nc.sync.dma_start(out=qn_sb, in_=qn_sb_d)

        ql2 = qlP.tile([128, 32, LORA], dk)  # part=(32a+b), free=(g, c)
        for G in range(8):
            wu_t = wuP.tile([NOPE, 4, 4, LORA], f8)
            for gg in range(4):
                nc.scalar.dma_start(out=wu_t[:, gg: gg + 1],
                                    in_=wu8[G][:, gg: gg + 1])
            for g4 in range(4):
                g = 4 * G + g4
                if g % 2 == 0:
                    ps = psS.tile([H, 512], f32, tag="ps", name=pname(f"psab{g}"))
                else:
                    ps = psX.tile([H, LORA], f32, tag="xps", name=pname(f"psab{g}"))
                for a in range(4):
                    # head h = 32a+g ; rows (a, b-pad32)
                    nc.tensor.matmul(
                        ps[32 * a: 32 * a + 32, :],
                        qn_sb[:, g, a, :],
                        wu_t[:, g4, a, :],
                        start=True, stop=True,
                        tile_position=(0, 32 * a),
                    )
                nc.vector.tensor_copy(ql2[:, g, :], ps)

        # roundtrip: ql_d[b, h, c] (32KB contiguous lines), per quadrant
        for a in range(4):
            nc.scalar.dma_start(
                out=ql_d.rearrange("b (a g) c -> a b g c", a=4)[a],
                in_=ql2[32 * a: 32 * a + BL],
            )

        # q lhsT tiles: lq[b] = [c(128), ck, h] via PE transpose of ql_d[b]
        lq = []
        lqro = []
        for b in range(BL):
            qh = qlP.tile([128, LORA], dk, tag=f"qh{b}", name=pname(f"qh{b}"))
            eng = nc.sync if b % 2 == 0 else nc.scalar
            eng.dma_start(out=qh, in_=ql_d[b])
            t_ = constp.tile([128, NKC, H], f8, tag=f"lq{b}", name=pname(f"lq{b}"))
            psl = psT.tile([128, LORA], dk, tag="pT_ps", name=pname(f"psl{b}"))
            for k in range(NKC):
                nc.tensor.transpose(
                    psl[:, 128 * k: 128 * (k + 1)],
                    qh[:, 128 * k: 128 * (k + 1)], identity,
                )
            nc.vector.tensor_copy(t_.rearrange("p k h -> p (k h)"), psl)
            lq.append(t_)
            r_ = constp.tile([ROPE, H], dk, tag=f"lqro{b}", name=pname(f"lqro{b}"))
            nc.scalar.dma_start(out=r_, in_=qpe_t[b])
            lqro.append(r_)
        stA.close()

        # wv prefetch: gpsimd stream after kt/ro; zone-gated on phase-A close
        wvP = stB.enter_context(tc.tile_pool(name=pname("wvP"), bufs=9))
        wv_sbs = []
        for o in range(16):
            wv_sb = wvP.tile([128, 8, NKC, V], dk, tag="wv", name=pname(f"wv{o}"))
            nc.gpsimd.dma_start(out=wv_sb, in_=wv2[o])
            wv_sbs.append(wv_sb)

        # ---------- Phase B: attention per batch ----------
        x_sb = [xP.tile([H, LORA], dk, tag=f"x{b}", name=pname(f"x{b}"))
                for b in range(BL)]
        # xT4[:, k, b, h] = x[b, h, 128k+p]
        xT4 = xtP.tile([128, NKC, BL, H], dk)

        def attention_batch(b):
            kt, ro = kts[b], ros[b]
            probs = prP.tile([H, T], dk)
            # t = 1024*j2grp + 8p + k  (interleaved for 8KB v-cache DMA lines)
            pview = probs.rearrange("h (j p k) -> h j k p", j=NJ, k=8)
            sums = stP.tile([128, NT], f32, tag="sums")
            for j in range(NT):
                ps = psS.tile([H, 512], f32)
                for k in range(NKC):
                    nc.tensor.matmul(
                        ps, lq[b][:, k, :], kt[:, k, 512 * j: 512 * (j + 1)],
                        start=(k == 0), stop=False,
                    )
                nc.tensor.matmul(ps, lqro[b], ro[:, 512 * j: 512 * (j + 1)],
                                 start=False, stop=True)
                nc.scalar.activation(
                    probs[:, 512 * j: 512 * (j + 1)], ps,
                    mybir.ActivationFunctionType.Exp,
                    bias=0.0, scale=SCALE,
                    accum_out=sums[:, j: j + 1],
                )
            ssum = stP.tile([128, 1], f32, tag="ssum")
            nc.vector.tensor_reduce(
                ssum, sums, axis=mybir.AxisListType.X, op=mybir.AluOpType.add)
            rs = stP.tile([128, 1], f32, tag="rs")
            nc.vector.reciprocal(rs, ssum)

            xps = psX.tile([H, LORA], f32, tag="xps", name=pname(f"xps{b}"))
            vts = []
            for jj2 in range(NJ):
                vt = vtP.tile([128, 8, LORA], dk)
                vsrc = v_c[b, 1024 * jj2: 1024 * (jj2 + 1), :].rearrange(
                    "(p k) c -> p k c", k=8)
                if jj2 < NJ - 1:
                    nc.sync.dma_start(out=vt[0:64], in_=vsrc[0:64])
                    nc.sync.dma_start(out=vt[64:128], in_=vsrc[64:128])
                else:
                    nc.sync.dma_start(out=vt[0:64], in_=vsrc[0:64])
                    nc.sync.dma_start(out=vt[64:127], in_=vsrc[64:127])
                    nc.sync.dma_start(
                        out=vt[127:128, 0:7, :],
                        in_=v_c[b, 1024 * jj2 + 1016: 1024 * jj2 + 1023, :])
                    nc.sync.dma_start(out=vt[127:128, 7, :],
                                      in_=newkv[b: b + 1, :])
                vts.append(vt)

            def emit_transpose_group(g):
                jj2, half = divmod(g, 2)
                pT_ps = psT.tile([128, 512], dk, tag="pT_ps")
                for k4 in range(4):
                    nc.tensor.transpose(
                        pT_ps[:, 128 * k4: 128 * (k4 + 1)],
                        pview[:, jj2, 4 * half + k4, :], identity)
                pT = pTP.tile([128, 512], dk)
                nc.vector.tensor_copy(pT, pT_ps)
                return pT

            pT_next = emit_transpose_group(0)
            for g in range(2 * NJ):
                jj2, half = divmod(g, 2)
                pT_cur = pT_next
                if g + 1 < 2 * NJ:
                    pT_next = emit_transpose_group(g + 1)
                for k4 in range(4):
                    j2 = 8 * jj2 + 4 * half + k4
                    nc.tensor.matmul(
                        xps, pT_cur[:, 128 * k4: 128 * (k4 + 1)],
                        vts[jj2][:, 4 * half + k4, :],
                        start=(j2 == 0), stop=(j2 == NTC - 1),
                    )
            nc.vector.tensor_scalar_mul(x_sb[b], xps, rs)

            # xT chunks for the output projection
            xt_ps = psT.tile([128, LORA], dk, tag="pT_ps")
            for k in range(NKC):
                nc.tensor.transpose(
                    xt_ps[:, 128 * k: 128 * (k + 1)],
                    x_sb[b][:, 128 * k: 128 * (k + 1)], identity)
            nc.vector.tensor_copy(
                xT4[:, :, b, :], xt_ps.rearrange("p (k h) -> p k h", k=NKC))

        for b in range(BL):
            attention_batch(b)

        # ---------- Phase D: out-proj ----------
        out_sb = outP.tile([128, 32, V], f32)  # part=(32a+b), free=(m,v); h=4m+a
        for o in range(16):  # octets of heads
            wv_sb = wv_sbs[o]
            pso = psS.tile([H, 512], f32, tag="ps")
            for mm in range(2):  # two groups of 4 heads per octet
                m = 2 * o + mm
                for a in range(4):
                    h = 4 * m + a
                    i = h - 8 * o
                    for k in range(NKC):
                        nc.tensor.matmul(
                            pso[32 * a: 32 * a + 4, 128 * mm: 128 * (mm + 1)],
                            xT4[:, k, :, h],
                            wv_sb[:, i, k, :],
                            start=(k == 0), stop=(k == NKC - 1),
                            tile_position=(0, 32 * a),
                        )
            for a in range(4):
                dst = out_sb[32 * a: 32 * a + 4, 2 * o: 2 * o + 2, :].rearrange(
                    "p m v -> p (m v)")
                if a % 2 == 0:
                    nc.vector.tensor_copy(dst, pso[32 * a: 32 * a + 4, 0:256])
                else:
                    nc.scalar.copy(dst, pso[32 * a: 32 * a + 4, 0:256])
        for a in range(4):
            nc.sync.dma_start(
                out=out_d.rearrange("b (m a) v -> b m a v", a=4)[:, :, a, :],
                in_=out_sb[32 * a: 32 * a + 4, :, :],
            )
        stB.close()


USE_BF16 = True


def _host_prep(q_nope, q_pe, compressed_kv, k_pe, position_ids, attention_mask,
               start_pos, ckv_cache, k_pe_cache, sin, cos, wkv_b, bf16=None):
    import concourse.mybir as mybir
    f8np = mybir.dt.np(mybir.dt.float8e4)
    dk = np.float16
    q_nope = np.asarray(q_nope, np.float32)
    q_pe = np.asarray(q_pe, np.float32)
    compressed_kv = np.asarray(compressed_kv, np.float32)
    k_pe = np.asarray(k_pe, np.float32)
    position_ids = np.asarray(position_ids)
    attention_mask = np.asarray(attention_mask)
    ckv_cache = np.asarray(ckv_cache, np.float32)
    k_pe_cache = np.asarray(k_pe_cache, np.float32)
    sin = np.asarray(sin, np.float32)
    cos = np.asarray(cos, np.float32)
    wkv_b = np.asarray(wkv_b, np.float32)

    assert int(start_pos) == START_POS, f"kernel compiled for start_pos={START_POS}"
    assert not np.any(attention_mask), "kernel assumes an all-zero attention mask"

    q_pe_rot, k_pe_rot = _host_rope(q_pe, k_pe, cos, sin, position_ids)
    k_pe_rot = k_pe_rot[:, 0, 0, :]                      # [B, 64]

    w = wkv_b.reshape(H, NOPE + V, LORA)
    w_ukv = w[:, :NOPE, :]                               # [H, 128, 512]
    # wu8[G, d, g4, a, c] = w_ukv[32a + 4G + g4, d, c]  (head h = 32a + g)
    wu8 = np.ascontiguousarray(
        w_ukv.reshape(4, 8, 4, NOPE, LORA).transpose(1, 3, 2, 0, 4)).astype(f8np)
    # wv2[o, p, i, k, v] = w_v[8o+i][c=128k+p, v]
    wv_t = w[:, NOPE:, :].transpose(0, 2, 1)             # [h, c, v]
    wv2 = np.ascontiguousarray(
        wv_t.reshape(16, 8, NKC, 128, V).transpose(0, 3, 1, 2, 4)).astype(dk)

    # transposed keys: [B, 576, 4096] fp8
    kT = np.empty((B, CR, T), np.float32)
    kT[:, :LORA, :START_POS] = ckv_cache[:, :START_POS].transpose(0, 2, 1)
    kT[:, LORA:, :START_POS] = k_pe_cache[:, :START_POS].transpose(0, 2, 1)
    kT[:, :LORA, START_POS] = compressed_kv[:, 0]
    kT[:, LORA:, START_POS] = k_pe_rot
    kT8 = kT.astype(f8np)

    in_maps = []
    for c in range(N_CORES):
        bs = slice(c * BL, (c + 1) * BL)
        # qn_t[d, g, a, b] = q_nope[bs[b], 32a+g, 0, d], zero-padded to b=32
        qn = np.zeros((NOPE, 32, 4, 32), dk)
        qn[:, :, :, :BL] = (
            q_nope[bs, :, 0, :]                          # [4, 128, 128] (b,h,d)
            .reshape(BL, 4, 32, NOPE)                    # (b, a, g, d)
            .transpose(3, 2, 1, 0)                       # (d, g, a, b)
        )
        in_maps.append({
            "k8": np.ascontiguousarray(kT8[bs]),
            "v_c": ckv_cache[bs].astype(dk),
            "newkv": compressed_kv[bs, 0, :].astype(dk),
            "qn_t": qn,
            "qpe_t": np.ascontiguousarray(
                q_pe_rot[bs, :, 0, :].transpose(0, 2, 1)).astype(dk),
            "wu8": wu8,
            "wv2": wv2,
        })
    return in_maps


_EXEC_CACHE = {}


def _get_executor(nc):
    """jit-once PJRT executor for the 8-core SPMD program (mirrors
    concourse.bass2jax.run_bass_via_pjrt, but reusable across calls)."""
    key = id(nc)
    if key in _EXEC_CACHE:
        return _EXEC_CACHE[key]

    import jax
    import concourse.mybir as mybir
    from concourse.bass2jax import (
        _bass_exec_p, install_neuronx_cc_hook, partition_id_tensor,
    )
    from jax.sharding import Mesh, PartitionSpec
    try:
        from jax.experimental.shard_map import shard_map
    except ImportError:  # newer jax
        from jax.shard_map import shard_map

    install_neuronx_cc_hook()
    assert nc.dbg_addr is None
    partition_name = nc.partition_id_tensor.name if nc.partition_id_tensor else None

    in_names, out_names, out_avals = [], [], []
    for alloc in nc.m.functions[0].allocations:
        if not isinstance(alloc, mybir.MemoryLocationSet):
            continue
        name = alloc.memorylocations[0].name
        if alloc.kind == "ExternalInput":
            if name != partition_name:
                in_names.append(name)
        elif alloc.kind == "ExternalOutput":
            out_names.append(name)
            out_avals.append(jax.core.ShapedArray(
                tuple(alloc.tensor_shape), mybir.dt.np(alloc.dtype)))
    n_params = len(in_names)
    all_names = in_names + out_names
    if partition_name is not None:
        all_names = all_names + [partition_name]

    def _body(*args):
        operands = list(args)
        if partition_name is not None:
            operands.append(partition_id_tensor())
        outs = _bass_exec_p.bind(
            *operands,
            out_avals=tuple(out_avals),
            in_names=tuple(all_names),
            out_names=tuple(out_names),
            lowering_input_output_aliases=(),
            sim_require_finite=True,
            sim_require_nnan=True,
            nc=nc,
        )
        return tuple(outs)

    devices = jax.devices()[:N_CORES]
    mesh = Mesh(np.asarray(devices), ("core",))
    donate = tuple(range(n_params, n_params + len(out_names)))
    fn = jax.jit(
        shard_map(
            _body, mesh=mesh,
            in_specs=(PartitionSpec("core"),) * (n_params + len(out_names)),
            out_specs=(PartitionSpec("core"),) * len(out_names),
            check_rep=False,
        ),
        donate_argnums=donate, keep_unused=True,
    )
    entry = (fn, mesh, in_names, out_names, out_avals, n_params)
    _EXEC_CACHE[key] = entry
    return entry


def _execute(nc, in_maps, device_args=None):
    """Run once; returns (list-of-per-core output dicts, device_args)."""
    import jax

    fn, mesh, in_names, out_names, out_avals, n_params = _get_executor(nc)
    if device_args is None:
        from jax.sharding import NamedSharding, PartitionSpec
        sh = NamedSharding(mesh, PartitionSpec("core"))
        concat_in = [
            np.concatenate([np.asarray(m[name]) for m in in_maps], axis=0)
            for name in in_names
        ]
        device_args = [jax.device_put(a, sh) for a in concat_in]
    zeros = [np.zeros((N_CORES * av.shape[0], *av.shape[1:]), av.dtype)
             for av in out_avals]
    outs = fn(*device_args, *zeros)
    outs = [np.asarray(o) for o in outs]
    results = [
        {name: outs[i].reshape(N_CORES, *out_avals[i].shape)[c]
         for i, name in enumerate(out_names)}
        for c in range(N_CORES)
    ]
    return results, device_args


def kernel(**inputs) -> np.ndarray:
    nc = _build_program(1, USE_BF16)
    in_maps = _host_prep(**inputs)
    results, _ = _execute(nc, in_maps)
    out = np.empty((B, 1, H, V), np.float32)
    for c in range(N_CORES):
        out[c * BL:(c + 1) * BL, 0] = results[c]["out_d"]
    return out


# revision 38
# speedup vs baseline: 1.0038x; 1.0038x over previous
"""DeepseekV3 MLA decode attention (B=32, H=128, q_len=1, T=4096) on 8 trn2 NeuronCores.

Strategy: batch-parallel over the 8 cores (4 batches/core). Per core, the full
absorbed-MLA decode runs on device:
  - absorb:   q_lat[b,h,c] = q_nope[b,h,:] @ w_ukv[h]   (PE, quadrant-packed),
              then DRAM-roundtrip to [b,h,c] + PE transpose -> lq[b][c,h] fp8
  - scores:   s[h,t] = lq^T kT  (fp8 x fp8 on PE, fp32 PSUM)
  - softmax:  no-max-pass exp (logits are tiny: |s*scale| < ~1), fused
              denominator accumulation on ACT, reciprocal on DVE
  - value:    x[h,c] = probs @ ckv (probs transposed on PE, fp16)
  - out-proj: out[b,h,v] = x[b,h,:] @ w_v[h]^T          (PE, quadrant-packed)
The score-side operands (kT cache, w_ukv, k_pe^T) stream in fp8e4m3 - softmax's
tiny-logit regime makes score-side quantization error negligible. The value
path (ckv, probs, w_v) stays fp16. Host prep is layout-only (RoPE of the
single query token, cache transpose, weight reshapes); all matmul FLOPs are
on device.
"""

import functools

import numpy as np

B, H, NOPE, ROPE, V, LORA = 32, 128, 128, 64, 128, 512
CACHE_LEN, START_POS = 4096, 4095
T = START_POS + 1            # 4096 keys after cache append
CR = LORA + ROPE             # 576 rows of the transposed key matrix
SCALE = float((NOPE + ROPE) ** -0.5)
N_CORES = 8
BL = B // N_CORES            # 4 batches per core
NT = T // 512                # 8 score chunks of 512
NTC = T // 128               # 32 value chunks of 128
NKC = LORA // 128            # 4 latent c-chunks of 128
NJ = T // 1024               # 4 DMA tiles of 1024 keys


def _interleave_to_half(x):
    *lead, d = x.shape
    return x.reshape(*lead, d // 2, 2).swapaxes(-1, -2).reshape(*lead, d)


def _rotate_half(x):
    d = x.shape[-1]
    return np.concatenate([-x[..., d // 2:], x[..., :d // 2]], axis=-1)


def _host_rope(q_pe, k_pe, cos, sin, position_ids):
    kv_seq_len = 1
    c = cos[:kv_seq_len][position_ids][:, None]  # [B,1,1,64]
    s = sin[:kv_seq_len][position_ids][:, None]
    q = _interleave_to_half(q_pe)
    k = _interleave_to_half(k_pe)
    return q * c + _rotate_half(q) * s, k * c + _rotate_half(k) * s


@functools.lru_cache(maxsize=4)
def _build_program(reps=1, bf16=True):
    import concourse.bacc as bacc
    import concourse.mybir as mybir
    import concourse.tile as tile
    from concourse.masks import make_identity

    f32 = mybir.dt.float32
    dk = mybir.dt.float16
    f8 = mybir.dt.float8e4
    nc = bacc.Bacc("TRN2", target_bir_lowering=False, debug=False)

    # ---- DRAM I/O ----
    k8 = nc.dram_tensor("k8", [BL, CR, T], f8, kind="ExternalInput").ap()
    v_c = nc.dram_tensor("v_c", [BL, CACHE_LEN, LORA], dk, kind="ExternalInput").ap()
    newkv = nc.dram_tensor("newkv", [BL, LORA], dk, kind="ExternalInput").ap()
    qn_t = nc.dram_tensor("qn_t", [NOPE, 32, 4, 32], dk, kind="ExternalInput").ap()
    qpe_t = nc.dram_tensor("qpe_t", [BL, ROPE, H], dk, kind="ExternalInput").ap()
    wu8 = nc.dram_tensor("wu8", [8, NOPE, 4, 4, LORA], f8, kind="ExternalInput").ap()
    wv2 = nc.dram_tensor("wv2", [16, 128, 8, NKC, V], dk, kind="ExternalInput").ap()
    out_d = nc.dram_tensor("out_d", [BL, H, V], f32, kind="ExternalOutput").ap()

    from contextlib import ExitStack

    with tile.TileContext(nc) as tc, ExitStack() as st0:
        constp = st0.enter_context(tc.tile_pool(name="const", bufs=1))
        dramp = st0.enter_context(tc.tile_pool(name="dram", bufs=1, space="DRAM"))

        identity = constp.tile([128, 128], dk)
        make_identity(nc, identity)
        identity8 = constp.tile([128, 128], f8)
        make_identity(nc, identity8)

        ql_d = dramp.tile([BL, H, LORA], f8)

        for _rep in range(reps):
            _emit_body(nc, tc, mybir, f32, dk, f8, _rep,
                       identity, identity8, qn_t, k8, v_c, newkv, qpe_t,
                       wu8, wv2, ql_d, out_d)

    nc.compile()
    return nc


def _emit_body(nc, tc, mybir, f32, dk, f8, rep,
               identity, identity8, qn_sb_d, k8, v_c, newkv, qpe_t, wu8, wv2,
               ql_d, out_d):
    from contextlib import ExitStack

    def pname(s):
        return f"{s}_r{rep}"

    with ExitStack() as stC:
        constp = stC.enter_context(tc.tile_pool(name=pname("perrep"), bufs=1))

        # Phase B/D pools first: their SBUF zones must not overlap phase-A
        # pools so cache prefetch DMAs start at t=0.
        stB = ExitStack()
        ktP = stB.enter_context(tc.tile_pool(name=pname("ktP"), bufs=3))
        roP = stB.enter_context(tc.tile_pool(name=pname("roP"), bufs=2))
        vtP = stB.enter_context(tc.tile_pool(name=pname("vtP"), bufs=4))
        prP = stB.enter_context(tc.tile_pool(name=pname("prP"), bufs=3))
        stP = stB.enter_context(tc.tile_pool(name=pname("stP"), bufs=2))
        pTP = stB.enter_context(tc.tile_pool(name=pname("pTP"), bufs=2))
        xP = stB.enter_context(tc.tile_pool(name=pname("xP"), bufs=1))
        xtP = stB.enter_context(tc.tile_pool(name=pname("xtP"), bufs=1))
        outP = stB.enter_context(tc.tile_pool(name=pname("outP"), bufs=1))
        psS = stB.enter_context(tc.tile_pool(name=pname("psS"), bufs=4, space="PSUM"))
        psT = stB.enter_context(tc.tile_pool(name=pname("psT"), bufs=2, space="PSUM"))
        psX = stB.enter_context(tc.tile_pool(name=pname("psX"), bufs=2, space="PSUM"))

        # kt/ro prefetch triggers (gpsimd stream, no upstream deps)
        kts, ros = [], []
        for b in range(BL):
            kt = ktP.tile([128, NKC, T], f8)
            src_ = k8[b, 0:LORA, :].rearrange("(k p) t -> p k t", p=128)
            for k in range(NKC):
                nc.gpsimd.dma_start(out=kt[:, k, :], in_=src_[:, k, :])
            ro = roP.tile([ROPE, T], f8)
            nc.gpsimd.dma_start(out=ro, in_=k8[b, LORA:CR, :])
            kts.append(kt)
            ros.append(ro)

        # ---------- Phase A: absorb q_nope @ w_ukv -> lq[b] = [c, h] ----------
        stA = ExitStack()
        wuP = stA.enter_context(tc.tile_pool(name=pname("wuP"), bufs=4))
        qlP = stA.enter_context(tc.tile_pool(name=pname("qlP"), bufs=1))

        qn_sb = qlP.tile([NOPE, 32, 4, 32], dk)
        nc.sync.dma_start(out=qn_sb, in_=qn_sb_d)

        ql2 = qlP.tile([128, 32, LORA], f8)  # part=(32a+b), free=(g, c)
        for G in range(8):
            wu_t = wuP.tile([NOPE, 4, 4, LORA], f8)
            for gg in range(4):
                nc.scalar.dma_start(out=wu_t[:, gg: gg + 1],
                                    in_=wu8[G][:, gg: gg + 1])
            for g4 in range(4):
                g = 4 * G + g4
                if g % 2 == 0:
                    ps = psS.tile([H, 512], f32, tag="ps", name=pname(f"psab{g}"))
                else:
                    ps = psX.tile([H, LORA], f32, tag="xps", name=pname(f"psab{g}"))
                for a in range(4):
                    # head h = 32a+g ; rows (a, b-pad32)
                    nc.tensor.matmul(
                        ps[32 * a: 32 * a + 32, :],
                        qn_sb[:, g, a, :],
                        wu_t[:, g4, a, :],
                        start=True, stop=True,
                        tile_position=(0, 32 * a),
                    )
                nc.vector.tensor_copy(ql2[:, g, :], ps)

        # roundtrip: ql_d[b, h, c] (32KB contiguous lines), per quadrant
        for a in range(4):
            nc.scalar.dma_start(
                out=ql_d.rearrange("b (a g) c -> a b g c", a=4)[a],
                in_=ql2[32 * a: 32 * a + BL],
            )

        # q lhsT tiles: lq[b] = [c(128), ck, h] via PE transpose of ql_d[b]
        lq = []
        lqro = []
        for b in range(BL):
            qh8 = qlP.tile([128, LORA], f8, tag=f"qh{b}", name=pname(f"qh8{b}"))
            eng = nc.sync if b % 2 == 0 else nc.scalar
            eng.dma_start(out=qh8, in_=ql_d[b])
            qh = qlP.tile([128, LORA], dk, tag="qh16", name=pname(f"qh{b}"))
            nc.vector.tensor_copy(qh, qh8)
            t_ = constp.tile([128, NKC, H], f8, tag=f"lq{b}", name=pname(f"lq{b}"))
            psl = psT.tile([128, LORA], dk, tag="pT_ps", name=pname(f"psl{b}"))
            for k in range(NKC):
                nc.tensor.transpose(
                    psl[:, 128 * k: 128 * (k + 1)],
                    qh[:, 128 * k: 128 * (k + 1)], identity,
                )
            nc.vector.tensor_copy(t_.rearrange("p k h -> p (k h)"), psl)
            lq.append(t_)
            r_ = constp.tile([ROPE, H], dk, tag=f"lqro{b}", name=pname(f"lqro{b}"))
            nc.scalar.dma_start(out=r_, in_=qpe_t[b])
            lqro.append(r_)
        stA.close()

        # wv prefetch: gpsimd stream after kt/ro; zone-gated on phase-A close
        wvP = stB.enter_context(tc.tile_pool(name=pname("wvP"), bufs=8))
        wv_sbs = []
        for o in range(16):
            wv_sb = wvP.tile([128, 8, NKC, V], dk, tag="wv", name=pname(f"wv{o}"))
            nc.gpsimd.dma_start(out=wv_sb, in_=wv2[o])
            wv_sbs.append(wv_sb)

        # ---------- Phase B: attention per batch ----------
        x_sb = [xP.tile([H, LORA], dk, tag=f"x{b}", name=pname(f"x{b}"))
                for b in range(BL)]
        # xT4[:, k, b, h] = x[b, h, 128k+p]
        xT4 = xtP.tile([128, NKC, BL, H], dk)

        def attention_batch(b):
            kt, ro = kts[b], ros[b]
            probs = prP.tile([H, T], dk)
            # t = 1024*j2grp + 8p + k  (interleaved for 8KB v-cache DMA lines)
            pview = probs.rearrange("h (j p k) -> h j k p", j=NJ, k=8)
            sums = stP.tile([128, NT], f32, tag="sums")
            for j in range(NT):
                ps = psS.tile([H, 512], f32)
                for k in range(NKC):
                    nc.tensor.matmul(
                        ps, lq[b][:, k, :], kt[:, k, 512 * j: 512 * (j + 1)],
                        start=(k == 0), stop=False,
                    )
                nc.tensor.matmul(ps, lqro[b], ro[:, 512 * j: 512 * (j + 1)],
                                 start=False, stop=True)
                nc.scalar.activation(
                    probs[:, 512 * j: 512 * (j + 1)], ps,
                    mybir.ActivationFunctionType.Exp,
                    bias=0.0, scale=SCALE,
                    accum_out=sums[:, j: j + 1],
                )
            ssum = stP.tile([128, 1], f32, tag="ssum")
            nc.vector.tensor_reduce(
                ssum, sums, axis=mybir.AxisListType.X, op=mybir.AluOpType.add)
            rs = stP.tile([128, 1], f32, tag="rs")
            nc.vector.reciprocal(rs, ssum)

            xps = psX.tile([H, LORA], f32, tag="xps", name=pname(f"xps{b}"))
            vts = []
            for jj2 in range(NJ):
                vt = vtP.tile([128, 8, LORA], dk)
                vsrc = v_c[b, 1024 * jj2: 1024 * (jj2 + 1), :].rearrange(
                    "(p k) c -> p k c", k=8)
                if jj2 < NJ - 1:
                    nc.sync.dma_start(out=vt[0:64], in_=vsrc[0:64])
                    nc.sync.dma_start(out=vt[64:128], in_=vsrc[64:128])
                else:
                    nc.sync.dma_start(out=vt[0:64], in_=vsrc[0:64])
                    nc.sync.dma_start(out=vt[64:127], in_=vsrc[64:127])
                    nc.sync.dma_start(
                        out=vt[127:128, 0:7, :],
                        in_=v_c[b, 1024 * jj2 + 1016: 1024 * jj2 + 1023, :])
                    nc.sync.dma_start(out=vt[127:128, 7, :],
                                      in_=newkv[b: b + 1, :])
                vts.append(vt)

            def emit_transpose_group(g):
                jj2, half = divmod(g, 2)
                pT_ps = psT.tile([128, 512], dk, tag="pT_ps")
                for k4 in range(4):
                    nc.tensor.transpose(
                        pT_ps[:, 128 * k4: 128 * (k4 + 1)],
                        pview[:, jj2, 4 * half + k4, :], identity)
                pT = pTP.tile([128, 512], dk)
                nc.vector.tensor_copy(pT, pT_ps)
                return pT

            pT_next = emit_transpose_group(0)
            for g in range(2 * NJ):
                jj2, half = divmod(g, 2)
                pT_cur = pT_next
                if g + 1 < 2 * NJ:
                    pT_next = emit_transpose_group(g + 1)
                for k4 in range(4):
                    j2 = 8 * jj2 + 4 * half + k4
                    nc.tensor.matmul(
                        xps, pT_cur[:, 128 * k4: 128 * (k4 + 1)],
                        vts[jj2][:, 4 * half + k4, :],
                        start=(j2 == 0), stop=(j2 == NTC - 1),
                    )
            nc.vector.tensor_scalar_mul(x_sb[b], xps, rs)

            # xT chunks for the output projection
            xt_ps = psT.tile([128, LORA], dk, tag="pT_ps")
            for k in range(NKC):
                nc.tensor.transpose(
                    xt_ps[:, 128 * k: 128 * (k + 1)],
                    x_sb[b][:, 128 * k: 128 * (k + 1)], identity)
            nc.vector.tensor_copy(
                xT4[:, :, b, :], xt_ps.rearrange("p (k h) -> p k h", k=NKC))

        for b in range(BL):
            attention_batch(b)

        # ---------- Phase D: out-proj ----------
        out_sb = outP.tile([128, 32, V], f32)  # part=(32a+b), free=(m,v); h=4m+a
        for o in range(16):  # octets of heads
            wv_sb = wv_sbs[o]
            pso = psS.tile([H, 512], f32, tag="ps")
            for mm in range(2):  # two groups of 4 heads per octet
                m = 2 * o + mm
                for a in range(4):
                    h = 4 * m + a
                    i = h - 8 * o
                    for k in range(NKC):
                        nc.tensor.matmul(
                            pso[32 * a: 32 * a + 4, 128 * mm: 128 * (mm + 1)],
                            xT4[:, k, :, h],
                            wv_sb[:, i, k, :],
                            start=(k == 0), stop=(k == NKC - 1),
                            tile_position=(0, 32 * a),
                        )
            for a in range(4):
                dst = out_sb[32 * a: 32 * a + 4, 2 * o: 2 * o + 2, :].rearrange(
                    "p m v -> p (m v)")
                if a % 2 == 0:
                    nc.vector.tensor_copy(dst, pso[32 * a: 32 * a + 4, 0:256])
                else:
                    nc.scalar.copy(dst, pso[32 * a: 32 * a + 4, 0:256])
        for a in range(4):
            nc.sync.dma_start(
                out=out_d.rearrange("b (m a) v -> b m a v", a=4)[:, :, a, :],
                in_=out_sb[32 * a: 32 * a + 4, :, :],
            )
        stB.close()


USE_BF16 = True


def _host_prep(q_nope, q_pe, compressed_kv, k_pe, position_ids, attention_mask,
               start_pos, ckv_cache, k_pe_cache, sin, cos, wkv_b, bf16=None):
    import concourse.mybir as mybir
    f8np = mybir.dt.np(mybir.dt.float8e4)
    dk = np.float16
    q_nope = np.asarray(q_nope, np.float32)
    q_pe = np.asarray(q_pe, np.float32)
    compressed_kv = np.asarray(compressed_kv, np.float32)
    k_pe = np.asarray(k_pe, np.float32)
    position_ids = np.asarray(position_ids)
    attention_mask = np.asarray(attention_mask)
    ckv_cache = np.asarray(ckv_cache, np.float32)
    k_pe_cache = np.asarray(k_pe_cache, np.float32)
    sin = np.asarray(sin, np.float32)
    cos = np.asarray(cos, np.float32)
    wkv_b = np.asarray(wkv_b, np.float32)

    assert int(start_pos) == START_POS, f"kernel compiled for start_pos={START_POS}"
    assert not np.any(attention_mask), "kernel assumes an all-zero attention mask"

    q_pe_rot, k_pe_rot = _host_rope(q_pe, k_pe, cos, sin, position_ids)
    k_pe_rot = k_pe_rot[:, 0, 0, :]                      # [B, 64]

    w = wkv_b.reshape(H, NOPE + V, LORA)
    w_ukv = w[:, :NOPE, :]                               # [H, 128, 512]
    # wu8[G, d, g4, a, c] = w_ukv[32a + 4G + g4, d, c]  (head h = 32a + g)
    wu8 = np.ascontiguousarray(
        w_ukv.reshape(4, 8, 4, NOPE, LORA).transpose(1, 3, 2, 0, 4)).astype(f8np)
    # wv2[o, p, i, k, v] = w_v[8o+i][c=128k+p, v]
    wv_t = w[:, NOPE:, :].transpose(0, 2, 1)             # [h, c, v]
    wv2 = np.ascontiguousarray(
        wv_t.reshape(16, 8, NKC, 128, V).transpose(0, 3, 1, 2, 4)).astype(dk)

    # transposed keys: [B, 576, 4096] fp8
    kT = np.empty((B, CR, T), np.float32)
    kT[:, :LORA, :START_POS] = ckv_cache[:, :START_POS].transpose(0, 2, 1)
    kT[:, LORA:, :START_POS] = k_pe_cache[:, :START_POS].transpose(0, 2, 1)
    kT[:, :LORA, START_POS] = compressed_kv[:, 0]
    kT[:, LORA:, START_POS] = k_pe_rot
    kT8 = kT.astype(f8np)

    in_maps = []
    for c in range(N_CORES):
        bs = slice(c * BL, (c + 1) * BL)
        # qn_t[d, g, a, b] = q_nope[bs[b], 32a+g, 0, d], zero-padded to b=32
        qn = np.zeros((NOPE, 32, 4, 32), dk)
        qn[:, :, :, :BL] = (
            q_nope[bs, :, 0, :]                          # [4, 128, 128] (b,h,d)
            .reshape(BL, 4, 32, NOPE)                    # (b, a, g, d)
            .transpose(3, 2, 1, 0)                       # (d, g, a, b)
        )
        in_maps.append({
            "k8": np.ascontiguousarray(kT8[bs]),
            "v_c": ckv_cache[bs].astype(dk),
            "newkv": compressed_kv[bs, 0, :].astype(dk),
            "qn_t": qn,
            "qpe_t": np.ascontiguousarray(
                q_pe_rot[bs, :, 0, :].transpose(0, 2, 1)).astype(dk),
            "wu8": wu8,
            "wv2": wv2,
        })
    return in_maps


_EXEC_CACHE = {}


def _get_executor(nc):
    """jit-once PJRT executor for the 8-core SPMD program (mirrors
    concourse.bass2jax.run_bass_via_pjrt, but reusable across calls)."""
    key = id(nc)
    if key in _EXEC_CACHE:
        return _EXEC_CACHE[key]

    import jax
    import concourse.mybir as mybir
    from concourse.bass2jax import (
        _bass_exec_p, install_neuronx_cc_hook, partition_id_tensor,
    )
    from jax.sharding import Mesh, PartitionSpec
    try:
        from jax.experimental.shard_map import shard_map
    except ImportError:  # newer jax
        from jax.shard_map import shard_map

    install_neuronx_cc_hook()
    assert nc.dbg_addr is None
    partition_name = nc.partition_id_tensor.name if nc.partition_id_tensor else None

    in_names, out_names, out_avals = [], [], []
    for alloc in nc.m.functions[0].allocations:
        if not isinstance(alloc, mybir.MemoryLocationSet):
            continue
        name = alloc.memorylocations[0].name
        if alloc.kind == "ExternalInput":
            if name != partition_name:
                in_names.append(name)
        elif alloc.kind == "ExternalOutput":
            out_names.append(name)
            out_avals.append(jax.core.ShapedArray(
                tuple(alloc.tensor_shape), mybir.dt.np(alloc.dtype)))
    n_params = len(in_names)
    all_names = in_names + out_names
    if partition_name is not None:
        all_names = all_names + [partition_name]

    def _body(*args):
        operands = list(args)
        if partition_name is not None:
            operands.append(partition_id_tensor())
        outs = _bass_exec_p.bind(
            *operands,
            out_avals=tuple(out_avals),
            in_names=tuple(all_names),
            out_names=tuple(out_names),
            lowering_input_output_aliases=(),
            sim_require_finite=True,
            sim_require_nnan=True,
            nc=nc,
        )
        return tuple(outs)

    devices = jax.devices()[:N_CORES]
    mesh = Mesh(np.asarray(devices), ("core",))
    donate = tuple(range(n_params, n_params + len(out_names)))
    fn = jax.jit(
        shard_map(
            _body, mesh=mesh,
            in_specs=(PartitionSpec("core"),) * (n_params + len(out_names)),
            out_specs=(PartitionSpec("core"),) * len(out_names),
            check_rep=False,
        ),
        donate_argnums=donate, keep_unused=True,
    )
    entry = (fn, mesh, in_names, out_names, out_avals, n_params)
    _EXEC_CACHE[key] = entry
    return entry


def _execute(nc, in_maps, device_args=None):
    """Run once; returns (list-of-per-core output dicts, device_args)."""
    import jax

    fn, mesh, in_names, out_names, out_avals, n_params = _get_executor(nc)
    if device_args is None:
        from jax.sharding import NamedSharding, PartitionSpec
        sh = NamedSharding(mesh, PartitionSpec("core"))
        concat_in = [
            np.concatenate([np.asarray(m[name]) for m in in_maps], axis=0)
            for name in in_names
        ]
        device_args = [jax.device_put(a, sh) for a in concat_in]
    zeros = [np.zeros((N_CORES * av.shape[0], *av.shape[1:]), av.dtype)
             for av in out_avals]
    outs = fn(*device_args, *zeros)
    outs = [np.asarray(o) for o in outs]
    results = [
        {name: outs[i].reshape(N_CORES, *out_avals[i].shape)[c]
         for i, name in enumerate(out_names)}
        for c in range(N_CORES)
    ]
    return results, device_args


def kernel(**inputs) -> np.ndarray:
    nc = _build_program(1, USE_BF16)
    in_maps = _host_prep(**inputs)
    results, _ = _execute(nc, in_maps)
    out = np.empty((B, 1, H, V), np.float32)
    for c in range(N_CORES):
        out[c * BL:(c + 1) * BL, 0] = results[c]["out_d"]
    return out


# revision 41
# speedup vs baseline: 1.0455x; 1.0416x over previous
"""DeepseekV3 MLA decode attention (B=32, H=128, q_len=1, T=4096) on 8 trn2 NeuronCores.

Strategy: batch-parallel over the 8 cores (4 batches/core). Per core, the full
absorbed-MLA decode runs on device:
  - absorb:   q_lat[b,h,c] = q_nope[b,h,:] @ w_ukv[h]   (PE, quadrant-packed),
              then DRAM-roundtrip to [b,h,c] + PE transpose -> lq[b][c,h] fp8
  - scores:   s[h,t] = lq^T kT  (fp8 x fp8 on PE, fp32 PSUM)
  - softmax:  no-max-pass exp (logits are tiny: |s*scale| < ~1), fused
              denominator accumulation on ACT, reciprocal on DVE
  - value:    x[h,c] = probs @ ckv (probs transposed on PE, fp16)
  - out-proj: out[b,h,v] = x[b,h,:] @ w_v[h]^T          (PE, quadrant-packed)
The score-side operands (kT cache, w_ukv, k_pe^T) stream in fp8e4m3 - softmax's
tiny-logit regime makes score-side quantization error negligible. The value
path (ckv, probs, w_v) stays fp16. Host prep is layout-only (RoPE of the
single query token, cache transpose, weight reshapes); all matmul FLOPs are
on device.
"""

import functools

import numpy as np

B, H, NOPE, ROPE, V, LORA = 32, 128, 128, 64, 128, 512
CACHE_LEN, START_POS = 4096, 4095
T = START_POS + 1            # 4096 keys after cache append
CR = LORA + ROPE             # 576 rows of the transposed key matrix
SCALE = float((NOPE + ROPE) ** -0.5)
N_CORES = 8
BL = B // N_CORES            # 4 batches per core
NT = T // 512                # 8 score chunks of 512
NTC = T // 128               # 32 value chunks of 128
NKC = LORA // 128            # 4 latent c-chunks of 128
NJ = T // 1024               # 4 DMA tiles of 1024 keys


def _interleave_to_half(x):
    *lead, d = x.shape
    return x.reshape(*lead, d // 2, 2).swapaxes(-1, -2).reshape(*lead, d)


def _rotate_half(x):
    d = x.shape[-1]
    return np.concatenate([-x[..., d // 2:], x[..., :d // 2]], axis=-1)


def _host_rope(q_pe, k_pe, cos, sin, position_ids):
    kv_seq_len = 1
    c = cos[:kv_seq_len][position_ids][:, None]  # [B,1,1,64]
    s = sin[:kv_seq_len][position_ids][:, None]
    q = _interleave_to_half(q_pe)
    k = _interleave_to_half(k_pe)
    return q * c + _rotate_half(q) * s, k * c + _rotate_half(k) * s


@functools.lru_cache(maxsize=4)
def _build_program(reps=1, bf16=True):
    import concourse.bacc as bacc
    import concourse.mybir as mybir
    import concourse.tile as tile
    from concourse.masks import make_identity

    f32 = mybir.dt.float32
    dk = mybir.dt.float16
    f8 = mybir.dt.float8e4
    nc = bacc.Bacc("TRN2", target_bir_lowering=False, debug=False)

    # ---- DRAM I/O ----
    k8 = nc.dram_tensor("k8", [BL, CR, T], f8, kind="ExternalInput").ap()
    v_c = nc.dram_tensor("v_c", [BL, CACHE_LEN, LORA], dk, kind="ExternalInput").ap()
    newkv = nc.dram_tensor("newkv", [BL, LORA], dk, kind="ExternalInput").ap()
    qn_t = nc.dram_tensor("qn_t", [NOPE, 32, 4, 32], dk, kind="ExternalInput").ap()
    qpe_t = nc.dram_tensor("qpe_t", [BL, ROPE, H], dk, kind="ExternalInput").ap()
    wu8 = nc.dram_tensor("wu8", [8, NOPE, 4, 4, LORA], f8, kind="ExternalInput").ap()
    wv2 = nc.dram_tensor("wv2", [16, 128, 8, NKC, V], dk, kind="ExternalInput").ap()
    out_d = nc.dram_tensor("out_d", [BL, H, V], f32, kind="ExternalOutput").ap()

    from contextlib import ExitStack

    with tile.TileContext(nc) as tc, ExitStack() as st0:
        constp = st0.enter_context(tc.tile_pool(name="const", bufs=1))
        dramp = st0.enter_context(tc.tile_pool(name="dram", bufs=1, space="DRAM"))

        identity = constp.tile([128, 128], dk)
        make_identity(nc, identity)
        identity8 = constp.tile([128, 128], f8)
        make_identity(nc, identity8)

        ql_d = dramp.tile([BL, H, LORA], f8)

        for _rep in range(reps):
            _emit_body(nc, tc, mybir, f32, dk, f8, _rep,
                       identity, identity8, qn_t, k8, v_c, newkv, qpe_t,
                       wu8, wv2, ql_d, out_d)

    nc.compile()
    return nc


def _emit_body(nc, tc, mybir, f32, dk, f8, rep,
               identity, identity8, qn_sb_d, k8, v_c, newkv, qpe_t, wu8, wv2,
               ql_d, out_d):
    from contextlib import ExitStack

    def pname(s):
        return f"{s}_r{rep}"

    with ExitStack() as stC:
        constp = stC.enter_context(tc.tile_pool(name=pname("perrep"), bufs=1))

        # Phase B/D pools first: their SBUF zones must not overlap phase-A
        # pools so cache prefetch DMAs start at t=0.
        stB = ExitStack()
        ktP = stB.enter_context(tc.tile_pool(name=pname("ktP"), bufs=3))
        roP = stB.enter_context(tc.tile_pool(name=pname("roP"), bufs=2))
        vtP = stB.enter_context(tc.tile_pool(name=pname("vtP"), bufs=4))
        prP = stB.enter_context(tc.tile_pool(name=pname("prP"), bufs=3))
        stP = stB.enter_context(tc.tile_pool(name=pname("stP"), bufs=2))
        pTP = stB.enter_context(tc.tile_pool(name=pname("pTP"), bufs=2))
        xP = stB.enter_context(tc.tile_pool(name=pname("xP"), bufs=1))
        xtP = stB.enter_context(tc.tile_pool(name=pname("xtP"), bufs=1))
        outP = stB.enter_context(tc.tile_pool(name=pname("outP"), bufs=1))
        psS = stB.enter_context(tc.tile_pool(name=pname("psS"), bufs=4, space="PSUM"))
        psT = stB.enter_context(tc.tile_pool(name=pname("psT"), bufs=2, space="PSUM"))
        psX = stB.enter_context(tc.tile_pool(name=pname("psX"), bufs=2, space="PSUM"))

        # kt/ro prefetch triggers (gpsimd stream, no upstream deps)
        kts, ros = [], []
        for b in range(BL):
            kt = ktP.tile([128, NKC, T], f8)
            src_ = k8[b, 0:LORA, :].rearrange("(k p) t -> p k t", p=128)
            for k in range(NKC):
                nc.gpsimd.dma_start(out=kt[:, k, :], in_=src_[:, k, :])
            ro = roP.tile([ROPE, T], f8)
            nc.gpsimd.dma_start(out=ro, in_=k8[b, LORA:CR, :])
            kts.append(kt)
            ros.append(ro)

        # ---------- Phase A: absorb q_nope @ w_ukv -> lq[b] = [c, h] ----------
        stA = ExitStack()
        wuP = stA.enter_context(tc.tile_pool(name=pname("wuP"), bufs=4))
        qlP = stA.enter_context(tc.tile_pool(name=pname("qlP"), bufs=1))

        qn_sb = qlP.tile([NOPE, 32, 4, 32], dk)
        nc.sync.dma_start(out=qn_sb, in_=qn_sb_d)

        ql2 = qlP.tile([128, 32, LORA], f8)  # part=(32a+b), free=(g, c)
        for G in range(8):
            wu_t = wuP.tile([NOPE, 4, 4, LORA], f8)
            for gg in range(4):
                nc.scalar.dma_start(out=wu_t[:, gg: gg + 1],
                                    in_=wu8[G][:, gg: gg + 1])
            for g4 in range(4):
                g = 4 * G + g4
                if g % 2 == 0:
                    ps = psS.tile([H, 512], f32, tag="ps", name=pname(f"psab{g}"))
                else:
                    ps = psX.tile([H, LORA], f32, tag="xps", name=pname(f"psab{g}"))
                for a in range(4):
                    # head h = 32a+g ; rows (a, b-pad32)
                    nc.tensor.matmul(
                        ps[32 * a: 32 * a + 32, :],
                        qn_sb[:, g, a, :],
                        wu_t[:, g4, a, :],
                        start=True, stop=True,
                        tile_position=(0, 32 * a),
                    )
                nc.vector.tensor_copy(ql2[:, g, :], ps)

        # roundtrip: ql_d[b, h, c] (32KB contiguous lines), per quadrant
        for a in range(4):
            nc.scalar.dma_start(
                out=ql_d.rearrange("b (a g) c -> a b g c", a=4)[a],
                in_=ql2[32 * a: 32 * a + BL],
            )

        # q lhsT tiles: lq[b] = [c(128), ck, h] via PE transpose of ql_d[b]
        lq = []
        lqro = []
        for b in range(BL):
            qh8 = qlP.tile([128, LORA], f8, tag=f"qh{b}", name=pname(f"qh8{b}"))
            eng = nc.sync if b % 2 == 0 else nc.scalar
            eng.dma_start(out=qh8, in_=ql_d[b])
            qh = qlP.tile([128, LORA], dk, tag="qh16", name=pname(f"qh{b}"))
            nc.vector.tensor_copy(qh, qh8)
            t_ = constp.tile([128, NKC, H], f8, tag=f"lq{b}", name=pname(f"lq{b}"))
            psl = psT.tile([128, LORA], dk, tag="pT_ps", name=pname(f"psl{b}"))
            for k in range(NKC):
                nc.tensor.transpose(
                    psl[:, 128 * k: 128 * (k + 1)],
                    qh[:, 128 * k: 128 * (k + 1)], identity,
                )
            nc.vector.tensor_copy(t_.rearrange("p k h -> p (k h)"), psl)
            lq.append(t_)
            r_ = constp.tile([ROPE, H], dk, tag=f"lqro{b}", name=pname(f"lqro{b}"))
            nc.scalar.dma_start(out=r_, in_=qpe_t[b])
            lqro.append(r_)
        stA.close()

        # wv prefetch: gpsimd stream after kt/ro; zone-gated on phase-A close
        wvP = stB.enter_context(tc.tile_pool(name=pname("wvP"), bufs=8))
        wv_sbs = []
        for o in range(16):
            wv_sb = wvP.tile([128, 8, NKC, V], dk, tag="wv", name=pname(f"wv{o}"))
            nc.gpsimd.dma_start(out=wv_sb, in_=wv2[o])
            wv_sbs.append(wv_sb)

        # ---------- Phase B: attention per batch ----------
        x_sb = [xP.tile([H, LORA], dk, tag=f"x{b}", name=pname(f"x{b}"))
                for b in range(BL)]
        # xT4[:, k, b, h] = x[b, h, 128k+p]
        xT4 = xtP.tile([128, NKC, BL, H], dk)

        def attention_batch(b):
            kt, ro = kts[b], ros[b]
            probs = prP.tile([H, T], dk)
            # t = 1024*j2grp + 8p + k  (interleaved for 8KB v-cache DMA lines)
            pview = probs.rearrange("h (j p k) -> h j k p", j=NJ, k=8)
            sums = stP.tile([128, NT], f32, tag="sums")
            for j in range(NT):
                ps = psS.tile([H, 512], f32)
                for k in range(NKC):
                    nc.tensor.matmul(
                        ps, lq[b][:, k, :], kt[:, k, 512 * j: 512 * (j + 1)],
                        start=(k == 0), stop=False,
                    )
                nc.tensor.matmul(ps, lqro[b], ro[:, 512 * j: 512 * (j + 1)],
                                 start=False, stop=True)
                nc.scalar.activation(
                    probs[:, 512 * j: 512 * (j + 1)], ps,
                    mybir.ActivationFunctionType.Exp,
                    bias=0.0, scale=SCALE,
                    accum_out=sums[:, j: j + 1],
                )
            ssum = stP.tile([128, 1], f32, tag="ssum")
            nc.vector.tensor_reduce(
                ssum, sums, axis=mybir.AxisListType.X, op=mybir.AluOpType.add)
            rs = stP.tile([128, 1], f32, tag="rs")
            nc.vector.reciprocal(rs, ssum)

            xps = psX.tile([H, LORA], f32, tag="xps", name=pname(f"xps{b}"))
            vts = []
            for jj2 in range(NJ):
                vt = vtP.tile([128, 8, LORA], dk)
                vsrc = v_c[b, 1024 * jj2: 1024 * (jj2 + 1), :].rearrange(
                    "(p k) c -> p k c", k=8)
                if jj2 < NJ - 1:
                    nc.sync.dma_start(out=vt[0:64], in_=vsrc[0:64])
                    nc.sync.dma_start(out=vt[64:128], in_=vsrc[64:128])
                else:
                    nc.sync.dma_start(out=vt[0:64], in_=vsrc[0:64])
                    nc.sync.dma_start(out=vt[64:127], in_=vsrc[64:127])
                    nc.sync.dma_start(
                        out=vt[127:128, 0:7, :],
                        in_=v_c[b, 1024 * jj2 + 1016: 1024 * jj2 + 1023, :])
                    nc.sync.dma_start(out=vt[127:128, 7, :],
                                      in_=newkv[b: b + 1, :])
                vts.append(vt)

            def emit_transpose_group(g):
                jj2, half = divmod(g, 2)
                pT_ps = psT.tile([128, 512], dk, tag="pT_ps")
                for k4 in range(4):
                    nc.tensor.transpose(
                        pT_ps[:, 128 * k4: 128 * (k4 + 1)],
                        pview[:, jj2, 4 * half + k4, :], identity)
                pT = pTP.tile([128, 512], dk)
                nc.vector.tensor_copy(pT, pT_ps)
                return pT

            pT_next = emit_transpose_group(0)
            for g in range(2 * NJ):
                jj2, half = divmod(g, 2)
                pT_cur = pT_next
                if g + 1 < 2 * NJ:
                    pT_next = emit_transpose_group(g + 1)
                for k4 in range(4):
                    j2 = 8 * jj2 + 4 * half + k4
                    nc.tensor.matmul(
                        xps, pT_cur[:, 128 * k4: 128 * (k4 + 1)],
                        vts[jj2][:, 4 * half + k4, :],
                        start=(j2 == 0), stop=(j2 == NTC - 1),
                    )
            nc.vector.tensor_scalar_mul(x_sb[b], xps, rs)

            # xT chunks for the output projection
            xt_ps = psT.tile([128, LORA], dk, tag="pT_ps")
            for k in range(NKC):
                nc.tensor.transpose(
                    xt_ps[:, 128 * k: 128 * (k + 1)],
                    x_sb[b][:, 128 * k: 128 * (k + 1)], identity)
            nc.vector.tensor_copy(
                xT4[:, :, b, :], xt_ps.rearrange("p (k h) -> p k h", k=NKC))

        for b in range(BL):
            attention_batch(b)

        # ---------- Phase D: out-proj ----------
        out_sb = outP.tile([128, 32, V], f32)  # part=(32a+b), free=(m,v); h=4m+a
        for o in range(16):  # octets of heads
            wv_sb = wv_sbs[o]
            pso = psS.tile([H, 512], f32, tag="ps")
            for mm in range(2):  # two groups of 4 heads per octet
                m = 2 * o + mm
                for a in range(4):
                    h = 4 * m + a
                    i = h - 8 * o
                    for k in range(NKC):
                        nc.tensor.matmul(
                            pso[32 * a: 32 * a + 4, 128 * mm: 128 * (mm + 1)],
                            xT4[:, k, :, h],
                            wv_sb[:, i, k, :],
                            start=(k == 0), stop=(k == NKC - 1),
                            tile_position=(0, 32 * a),
                        )
            for a in range(4):
                dst = out_sb[32 * a: 32 * a + 4, 2 * o: 2 * o + 2, :].rearrange(
                    "p m v -> p (m v)")
                if a % 2 == 0:
                    nc.vector.tensor_copy(dst, pso[32 * a: 32 * a + 4, 0:256])
                else:
                    nc.scalar.copy(dst, pso[32 * a: 32 * a + 4, 0:256])
        for a in range(4):
            nc.sync.dma_start(
                out=out_d.rearrange("b (m a) v -> b m a v", a=4)[:, :, a, :],
                in_=out_sb[32 * a: 32 * a + 4, :, :],
            )
        stB.close()


USE_BF16 = True


def _host_prep(q_nope, q_pe, compressed_kv, k_pe, position_ids, attention_mask,
               start_pos, ckv_cache, k_pe_cache, sin, cos, wkv_b, bf16=None):
    import concourse.mybir as mybir
    f8np = mybir.dt.np(mybir.dt.float8e4)
    dk = np.float16
    q_nope = np.asarray(q_nope, np.float32)
    q_pe = np.asarray(q_pe, np.float32)
    compressed_kv = np.asarray(compressed_kv, np.float32)
    k_pe = np.asarray(k_pe, np.float32)
    position_ids = np.asarray(position_ids)
    attention_mask = np.asarray(attention_mask)
    ckv_cache = np.asarray(ckv_cache, np.float32)
    k_pe_cache = np.asarray(k_pe_cache, np.float32)
    sin = np.asarray(sin, np.float32)
    cos = np.asarray(cos, np.float32)
    wkv_b = np.asarray(wkv_b, np.float32)

    assert int(start_pos) == START_POS, f"kernel compiled for start_pos={START_POS}"
    assert not np.any(attention_mask), "kernel assumes an all-zero attention mask"

    q_pe_rot, k_pe_rot = _host_rope(q_pe, k_pe, cos, sin, position_ids)
    k_pe_rot = k_pe_rot[:, 0, 0, :]                      # [B, 64]

    w = wkv_b.reshape(H, NOPE + V, LORA)
    w_ukv = w[:, :NOPE, :]                               # [H, 128, 512]
    # wu8[G, d, g4, a, c] = w_ukv[32a + 4G + g4, d, c]  (head h = 32a + g)
    wu8 = np.ascontiguousarray(
        w_ukv.reshape(4, 8, 4, NOPE, LORA).transpose(1, 3, 2, 0, 4)).astype(f8np)
    # wv2[o, p, i, k, v] = w_v[8o+i][c=128k+p, v]
    wv_t = w[:, NOPE:, :].transpose(0, 2, 1)             # [h, c, v]
    wv2 = np.ascontiguousarray(
        wv_t.reshape(16, 8, NKC, 128, V).transpose(0, 3, 1, 2, 4)).astype(dk)

    # transposed keys: [B, 576, 4096] fp8
    kT = np.empty((B, CR, T), np.float32)
    kT[:, :LORA, :START_POS] = ckv_cache[:, :START_POS].transpose(0, 2, 1)
    kT[:, LORA:, :START_POS] = k_pe_cache[:, :START_POS].transpose(0, 2, 1)
    kT[:, :LORA, START_POS] = compressed_kv[:, 0]
    kT[:, LORA:, START_POS] = k_pe_rot
    kT8 = kT.astype(f8np)

    in_maps = []
    for c in range(N_CORES):
        bs = slice(c * BL, (c + 1) * BL)
        # qn_t[d, g, a, b] = q_nope[bs[b], 32a+g, 0, d], zero-padded to b=32
        qn = np.zeros((NOPE, 32, 4, 32), dk)
        qn[:, :, :, :BL] = (
            q_nope[bs, :, 0, :]                          # [4, 128, 128] (b,h,d)
            .reshape(BL, 4, 32, NOPE)                    # (b, a, g, d)
            .transpose(3, 2, 1, 0)                       # (d, g, a, b)
        )
        in_maps.append({
            "k8": np.ascontiguousarray(kT8[bs]),
            "v_c": ckv_cache[bs].astype(dk),
            "newkv": compressed_kv[bs, 0, :].astype(dk),
            "qn_t": qn,
            "qpe_t": np.ascontiguousarray(
                q_pe_rot[bs, :, 0, :].transpose(0, 2, 1)).astype(dk),
            "wu8": wu8,
            "wv2": wv2,
        })
    return in_maps


_EXEC_CACHE = {}


def _get_executor(nc):
    """jit-once PJRT executor for the 8-core SPMD program (mirrors
    concourse.bass2jax.run_bass_via_pjrt, but reusable across calls)."""
    key = id(nc)
    if key in _EXEC_CACHE:
        return _EXEC_CACHE[key]

    import jax
    import concourse.mybir as mybir
    from concourse.bass2jax import (
        _bass_exec_p, install_neuronx_cc_hook, partition_id_tensor,
    )
    from jax.sharding import Mesh, PartitionSpec
    try:
        from jax.experimental.shard_map import shard_map
    except ImportError:  # newer jax
        from jax.shard_map import shard_map

    install_neuronx_cc_hook()
    assert nc.dbg_addr is None
    partition_name = nc.partition_id_tensor.name if nc.partition_id_tensor else None

    in_names, out_names, out_avals = [], [], []
    for alloc in nc.m.functions[0].allocations:
        if not isinstance(alloc, mybir.MemoryLocationSet):
            continue
        name = alloc.memorylocations[0].name
        if alloc.kind == "ExternalInput":
            if name != partition_name:
                in_names.append(name)
        elif alloc.kind == "ExternalOutput":
            out_names.append(name)
            out_avals.append(jax.core.ShapedArray(
                tuple(alloc.tensor_shape), mybir.dt.np(alloc.dtype)))
    n_params = len(in_names)
    all_names = in_names + out_names
    if partition_name is not None:
        all_names = all_names + [partition_name]

    def _body(*args):
        operands = list(args)
        if partition_name is not None:
            operands.append(partition_id_tensor())
        outs = _bass_exec_p.bind(
            *operands,
            out_avals=tuple(out_avals),
            in_names=tuple(all_names),
            out_names=tuple(out_names),
            lowering_input_output_aliases=(),
            sim_require_finite=True,
            sim_require_nnan=True,
            nc=nc,
        )
        return tuple(outs)

    devices = jax.devices()[:N_CORES]
    mesh = Mesh(np.asarray(devices), ("core",))
    donate = tuple(range(n_params, n_params + len(out_names)))
    fn = jax.jit(
        shard_map(
            _body, mesh=mesh,
            in_specs=(PartitionSpec("core"),) * (n_params + len(out_names)),
            out_specs=(PartitionSpec("core"),) * len(out_names),
            check_rep=False,
        ),
        donate_argnums=donate, keep_unused=True,
    )
    entry = (fn, mesh, in_names, out_names, out_avals, n_params)
    _EXEC_CACHE[key] = entry
    return entry


def _execute(nc, in_maps, device_args=None):
    """Run once; returns (list-of-per-core output dicts, device_args)."""
    import jax

    fn, mesh, in_names, out_names, out_avals, n_params = _get_executor(nc)
    if device_args is None:
        from jax.sharding import NamedSharding, PartitionSpec
        sh = NamedSharding(mesh, PartitionSpec("core"))
        concat_in = [
            np.concatenate([np.asarray(m[name]) for m in in_maps], axis=0)
            for name in in_names
        ]
        device_args = [jax.device_put(a, sh) for a in concat_in]
    zeros = [np.zeros((N_CORES * av.shape[0], *av.shape[1:]), av.dtype)
             for av in out_avals]
    outs = fn(*device_args, *zeros)
    outs = [np.asarray(o) for o in outs]
    results = [
        {name: outs[i].reshape(N_CORES, *out_avals[i].shape)[c]
         for i, name in enumerate(out_names)}
        for c in range(N_CORES)
    ]
    return results, device_args


def kernel(**inputs) -> np.ndarray:
    nc = _build_program(1, USE_BF16)
    in_maps = _host_prep(**inputs)
    results, _ = _execute(nc, in_maps)
    out = np.empty((B, 1, H, V), np.float32)
    for c in range(N_CORES):
        out[c * BL:(c + 1) * BL, 0] = results[c]["out_d"]
    return out
